# revision 20
# baseline (speedup 1.0000x reference)
"""Trainium2 Bass kernel for a YOLO-style detection loss.

Sharding: data-parallel over batch — 8 NeuronCores, 4 batches/core.
Per-core partial sums land in a [128, 16] tile; the host sums the
relevant slices of the 8 tiles and assembles the 4 scalar losses
(this host gather replaces the all-reduce of 4 scalars).

Key observation: the loss only touches pred densely through the
objectness channel (BCE vs 0 over every cell).  The class BCE term
needs the 80 class logits only at the assigned cells, and the box term
needs channels 0..3 there.  Device work:

1. OBJ stream: softplus over channel 4 of every cell (one [128, 263]
   tile), per-scale sums via DVE column reduces; the positive-cell
   correction (BCE(x,1)-BCE(x,0) = -x) comes from the gathered rows.
2. One 85-float channels-last row gather per target (indirect DMA;
   contiguous rows of a host-transposed [cells, 85] copy; 128 rows per
   call, 3 calls for up to 384 targets/core), then ~25 small DVE/ACT
   ops: box decode + l1, per-scale positive sums, class softplus sums,
   and the target-class logit correction as a one-hot dot product.

softplus(x) = ln(exp(x) + 1); Exp/Ln/Abs are pinned to the single ACT
table that holds all three (natural_log_exp_and_others) to avoid
per-instruction table reloads.  Sigmoid = 1/(1+exp(-x)) via DVE
reciprocal.  tensor_tensor_reduce is broken on this HW build, so
reductions use multiply + tensor_reduce.
"""

import numpy as np

from concourse import bass, bacc, mybir
from concourse import bass_utils
from concourse.tile import TileContext

F32 = mybir.dt.float32
I32 = mybir.dt.int32

NUM_CLASSES = 80
STAL_GAMMA = np.float32(2.0)
BATCH = 32
NCORES = 8
BPC = BATCH // NCORES          # batches per core
CH = 5 + NUM_CLASSES
HW = (80 * 80, 40 * 40, 20 * 20)
WS = (80, 40, 20)
NCELL = BPC * (HW[0] + HW[1] + HW[2])       # 33600 cells per core
COFF = (0, BPC * HW[0], BPC * (HW[0] + HW[1]))  # per-scale cell offsets
# OBJ stream: per-scale column blocks, scale 2 padded to 128*13
OBJ_COLS = (HW[0] * BPC // 128, HW[1] * BPC // 128, 1664 // 128)  # 200,50,13
NOBJ = HW[0] * BPC + HW[1] * BPC + 1664     # 33664 (64 pad cells of -100)
GROUPS = 3                                  # gather calls (128 targets each)
TPAD = 128 * GROUPS                         # 384; mean load is ~256/core
# meta column layout (GROUPS target-columns per quantity, interleaved)
MC_ADD = 0                                  # (gx, gy)          6 cols
MC_MUL = 6                                  # 1/w x4           12 cols
MC_SUB = 18                                 # (cx, cy, bw, bh) 12 cols
MC_SWM = 30                                 # small_weight/4    3 cols
MC_D0 = 33                                  # obj dedup flags   9 cols
MC_VLD = 42                                 # real-target flag  3 cols
MC_GI = 45                                  # gather row offsets (i32 bits)
MC_OH = 48                                  # class one-hot   240 cols
NMETA = MC_OH + GROUPS * NUM_CLASSES        # 288
# output partial tile column layout
OC_WSP = 0      # class softplus-sum term
OC_OBJ = 1      # 3 cols: per-scale objectness softplus sums
OC_BOX = 4
OC_POS = 5      # 3 cols
OC_CORR = 8
NOUT = 16

_NC_CACHE = None


def _ap(handle_ap, off, dims):
    return bass.AP(handle_ap.tensor, off, [list(d) for d in dims])


def _single_act_table(arch):
    """All of Exp/Ln/Abs live in natural_log_exp_and_others; hide them
    from the other tables so every activation uses one table (one load
    instead of a reload on each Exp<->Ln transition)."""
    tabs = _ORIG_TABLES(arch)
    need = {mybir.ActivationFunctionType.Exp,
            mybir.ActivationFunctionType.Ln}
    out = {}
    for name, fns in tabs.items():
        out[name] = fns if name == "natural_log_exp_and_others" \
            else fns - need
    return out


_ORIG_TABLES = bacc.get_activation_tables


def _build_nc():
    nc = bacc.Bacc("TRN2", target_bir_lowering=False, debug=False)
    fall_t = nc.dram_tensor("FALL", [NCELL * CH], F32, kind="ExternalInput")
    obj_t = nc.dram_tensor("OBJ", [128, sum(OBJ_COLS)], F32,
                           kind="ExternalInput")
    mt_t = nc.dram_tensor("MT", [128, NMETA], F32, kind="ExternalInput")
    out_t = nc.dram_tensor("OUT", [128, NOUT], F32, kind="ExternalOutput")

    EXP = mybir.ActivationFunctionType.Exp
    LN = mybir.ActivationFunctionType.Ln
    AX = mybir.AxisListType
    NOB = sum(OBJ_COLS)
    with TileContext(nc) as tc:
        with tc.tile_pool(name="persist", bufs=1) as pp:
            part = pp.tile([128, NOUT], F32)
            nc.vector.memset(part[:], 0.0)

            mt = pp.tile([128, NMETA], F32)
            va = pp.tile([128, GROUPS * CH], F32)  # per-target 85-float rows
            vt = pp.tile([128, GROUPS * NUM_CLASSES], F32)
            l1 = pp.tile([128, GROUPS], F32)
            sc = pp.tile([128, GROUPS], F32)
            g3 = pp.tile([128, GROUPS], F32)
            ob = pp.tile([128, NOB], F32)
            # meta (with bit-packed gather offsets) on the scalar HWDGE
            # ring, objectness on the sync ring - they run in parallel
            nc.scalar.dma_start(out=mt[:], in_=mt_t.ap())
            nc.sync.dma_start(out=ob[:], in_=obj_t.ap())
            gi = mt[:, MC_GI:MC_GI + GROUPS].bitcast(I32)
            # one 85-float row per target; 128 rows (one per partition)
            # per call; target t sits at (p, j) = (t % 128, t // 128)
            for j in range(GROUPS):
                nc.gpsimd.indirect_dma_start(
                    out=va[:, CH * j:CH * j + CH], out_offset=None,
                    in_=_ap(fall_t.ap(), 0, [[1, NCELL * CH], [1, 1]]),
                    in_offset=bass.IndirectOffsetOnAxis(ap=gi[:, j:j + 1],
                                                        axis=0))

            # ---- dense objectness stream ----
            nc.scalar.activation(ob[:], ob[:], EXP)
            nc.scalar.activation(ob[:], ob[:], LN, bias=1.0)
            ocol = 0
            for s in range(3):
                w = OBJ_COLS[s]
                nc.vector.reduce_sum(part[:, OC_OBJ + s:OC_OBJ + s + 1],
                                     ob[:, ocol:ocol + w], axis=AX.X)
                ocol += w

            # ---- per-target math ----
            va3 = va[:].rearrange("p (j c) -> p j c", c=CH)
            vt3 = vt[:].rearrange("p (j c) -> p j c", c=NUM_CLASSES)
            mt3 = lambda lo, w: mt[:, lo:lo + GROUPS * w].rearrange(
                "p (j c) -> p j c", c=w)
            # class-logit correction: one-hot dot with the raw logits
            nc.vector.tensor_mul(vt3, va3[:, :, 5:CH], mt3(MC_OH, NUM_CLASSES))
            nc.vector.reduce_sum(part[:, OC_CORR:OC_CORR + 1], vt[:],
                                 axis=AX.X)
            # box decode: ch0,1 -> sigmoid = 1/(1+exp(-x)) ; ch2,3 ->
            # exp(min(x,4)); one shared EXP pass over ch0..3
            nc.vector.tensor_scalar_mul(va3[:, :, 0:2], va3[:, :, 0:2], -1.0)
            nc.vector.tensor_scalar_min(va3[:, :, 2:4], va3[:, :, 2:4], 4.0)
            nc.scalar.activation(va3[:, :, 0:4], va3[:, :, 0:4], EXP)
            nc.vector.tensor_scalar_add(va3[:, :, 0:2], va3[:, :, 0:2], 1.0)
            nc.vector.reciprocal(va3[:, :, 0:2], va3[:, :, 0:2])
            nc.vector.tensor_add(va3[:, :, 0:2], va3[:, :, 0:2], mt3(MC_ADD, 2))
            nc.vector.tensor_mul(va3[:, :, 0:4], va3[:, :, 0:4], mt3(MC_MUL, 4))
            nc.vector.tensor_sub(va3[:, :, 0:4], va3[:, :, 0:4], mt3(MC_SUB, 4))
            nc.vector.reduce_sum(l1[:], va3[:, :, 0:4], axis=AX.X,
                                 apply_absolute_value=True)
            nc.vector.tensor_mul(l1[:], l1[:], mt[:, MC_SWM:MC_SWM + GROUPS])
            nc.vector.reduce_sum(part[:, OC_BOX:OC_BOX + 1], l1[:], axis=AX.X)
            # objectness positive-cell correction (raw channel 4)
            for s in range(3):
                nc.vector.tensor_mul(
                    sc[:], va3[:, :, 4],
                    mt[:, MC_D0 + GROUPS * s:MC_D0 + GROUPS * s + GROUPS])
                nc.vector.reduce_sum(part[:, OC_POS + s:OC_POS + s + 1],
                                     sc[:], axis=AX.X)
            # class softplus sum over the 80 logits of each target's cell
            nc.scalar.activation(va3[:, :, 5:CH], va3[:, :, 5:CH], EXP)
            nc.scalar.activation(va3[:, :, 5:CH], va3[:, :, 5:CH], LN,
                                 bias=1.0)
            nc.vector.reduce_sum(g3[:], va3[:, :, 5:CH], axis=AX.X)
            nc.vector.tensor_mul(g3[:], g3[:], mt[:, MC_VLD:MC_VLD + GROUPS])
            nc.vector.reduce_sum(part[:, OC_WSP:OC_WSP + 1], g3[:], axis=AX.X)

            nc.sync.dma_start(out=out_t.ap(), in_=part[:])
    bacc.get_activation_tables = _single_act_table
    try:
        nc.compile()
    finally:
        bacc.get_activation_tables = _ORIG_TABLES
    return nc


def get_nc():
    global _NC_CACHE
    if _NC_CACHE is None:
        _NC_CACHE = _build_nc()
    return _NC_CACHE


def prepare_in_maps(pred0, pred1, pred2, targets):
    """Host-side sharding + layout/index preprocessing (numpy only)."""
    preds = (np.asarray(pred0, dtype=np.float32),
             np.asarray(pred1, dtype=np.float32),
             np.asarray(pred2, dtype=np.float32))
    t = np.asarray(targets, dtype=np.float32)
    n = t.shape[0]
    b = t[:, 0].astype(np.int32)
    cls = t[:, 1].astype(np.int32)
    cx, cy, bw, bh = t[:, 2], t[:, 3], t[:, 4], t[:, 5]

    area = np.maximum(bw * bh, np.float32(1e-6))
    s_idx = np.where(area <= 0.01, 0,
                     np.where(area <= 0.03, 1, 2)).astype(np.int32)
    sw = np.float32(1.0) + STAL_GAMMA * (np.float32(1.0) - np.sqrt(area))

    ws = np.array(WS, np.int32)[s_idx]
    wf = ws.astype(np.float32)
    gx = np.clip((cx * wf).astype(np.int32), 0, ws - 1)
    gy = np.clip((cy * wf).astype(np.int32), 0, ws - 1)
    hw = np.array(HW, np.int64)[s_idx]

    b_cl = np.clip(b, 0, BATCH - 1)
    core = b_cl // BPC
    bl = (b_cl % BPC).astype(np.int64)
    cell = (np.array(COFF, np.int64)[s_idx] + bl * hw
            + (gy.astype(np.int64) * ws + gx))

    valid_cls = ((cls >= 0) & (cls < NUM_CLASSES)).astype(np.float32)
    cls_c = np.clip(cls, 0, NUM_CLASSES - 1)

    # obj dedup: one representative target per (scale, batch, gy, gx) cell
    key = ((s_idx.astype(np.int64) * BATCH + b_cl) * 128 + gy) * 128 + gx
    dflag = np.zeros(n, np.float32)
    _, first = np.unique(key, return_index=True)
    dflag[first] = 1.0

    in_maps = []
    for c in range(NCORES):
        sel = np.nonzero(core == c)[0]
        if len(sel) > TPAD:
            sel = sel[:TPAD]  # graceful degradation; never expected
        m = len(sel)
        csel = cell[sel]

        # target t maps to (partition, group) = (t % 128, t // 128)
        ga = np.zeros(TPAD, np.int64)
        ga[:m] = csel * CH

        mt = np.zeros((128, NMETA), np.float32)
        mt[:, MC_GI:MC_GI + GROUPS] = np.ascontiguousarray(
            ga.astype(np.int32).reshape(GROUPS, 128).T).view(np.float32)

        def put(col, vals):
            buf = np.zeros(TPAD, np.float32)
            buf[:m] = vals
            mt[:, col:col + GROUPS] = buf.reshape(GROUPS, 128).T

        def put_il(col, width, *vals):  # channel-interleaved group
            buf = np.zeros((TPAD, width), np.float32)
            for i, v in enumerate(vals):
                buf[:m, i] = v
            mt[:, col:col + GROUPS * width] = buf.reshape(
                GROUPS, 128, width).transpose(1, 0, 2).reshape(
                128, GROUPS * width)

        invw = np.float32(1.0) / wf[sel]
        put_il(MC_ADD, 2, gx[sel].astype(np.float32),
               gy[sel].astype(np.float32))
        put_il(MC_MUL, 4, invw, invw, invw, invw)
        put_il(MC_SUB, 4, cx[sel], cy[sel], bw[sel], bh[sel])
        put(MC_SWM, sw[sel] * np.float32(0.25))
        for s in range(3):
            put(MC_D0 + GROUPS * s, dflag[sel] * (s_idx[sel] == s))
        put(MC_VLD, np.float32(1.0))
        oh = np.zeros((TPAD, NUM_CLASSES), np.float32)
        oh[np.arange(m), cls_c[sel]] = valid_cls[sel]
        mt[:, MC_OH:] = oh.reshape(GROUPS, 128, NUM_CLASSES).transpose(
            1, 0, 2).reshape(128, GROUPS * NUM_CLASSES)

        lo, hi = c * BPC, (c + 1) * BPC
        fall = np.empty((NCELL, CH), np.float32)
        obj = np.full((128, sum(OBJ_COLS)), np.float32(-100.0), np.float32)
        off = 0
        ocol = 0
        for s, p in enumerate(preds):
            nc_s = BPC * HW[s]
            blk = p[lo:hi].reshape(BPC, CH, HW[s])
            fall[off:off + nc_s] = np.moveaxis(blk, 1, 2).reshape(nc_s, CH)
            w = OBJ_COLS[s]
            tmp = np.full(128 * w, np.float32(-100.0), np.float32)
            tmp[:nc_s] = blk[:, 4].reshape(-1)
            obj[:, ocol:ocol + w] = tmp.reshape(128, w)
            off += nc_s
            ocol += w

        in_maps.append({
            "FALL": fall.reshape(-1),
            "OBJ": obj,
            "MT": mt,
        })
    return in_maps, n


def finalize(results, n):
    """Combine per-core [128, NOUT] partial tiles into the 4 losses."""
    ps = np.stack([np.asarray(r["OUT"], np.float64) for r in results])
    cls_sp = ps[:, :, OC_WSP].sum()
    obj_sp = [ps[:, :, OC_OBJ + s].sum() for s in range(3)]
    box = ps[:, :, OC_BOX].sum()
    pos = [ps[:, :, OC_POS + s].sum() for s in range(3)]
    corr = ps[:, :, OC_CORR].sum()

    norm = max(1, n)
    box_loss = box / norm
    cls_loss = (cls_sp - corr) / (NUM_CLASSES * norm)
    obj_loss = sum((obj_sp[s] - pos[s]) / (BATCH * HW[s]) for s in range(3))
    total = box_loss + obj_loss + cls_loss
    return np.array([total, box_loss, obj_loss, cls_loss], np.float32)


def run_on_hw(in_maps, trace=False):
    nc = get_nc()
    return bass_utils.run_bass_kernel_spmd(
        nc, in_maps, core_ids=list(range(NCORES)), trace=trace)


def kernel(pred0, pred1, pred2, targets, **_unused):
    in_maps, n = prepare_in_maps(pred0, pred1, pred2, targets)
    res = run_on_hw(in_maps)
    return finalize(res.results, n)


# revision 21
# speedup vs baseline: 1.0297x; 1.0297x over previous
"""Trainium2 Bass kernel for a YOLO-style detection loss.

Sharding: data-parallel over batch — 8 NeuronCores, 4 batches/core.
Per-core partial sums land in a [128, 16] tile; the host sums the
relevant slices of the 8 tiles and assembles the 4 scalar losses
(this host gather replaces the all-reduce of 4 scalars).

Key observation: the loss only touches pred densely through the
objectness channel (BCE vs 0 over every cell).  The class BCE term
needs the 80 class logits only at the assigned cells, and the box term
needs channels 0..3 there.  Device work:

1. OBJ stream: softplus over channel 4 of every cell (one [128, 263]
   tile), per-scale sums via DVE column reduces; the positive-cell
   correction (BCE(x,1)-BCE(x,0) = -x) comes from the gathered rows.
2. One 85-float channels-last row gather per target (indirect DMA;
   contiguous rows of a host-transposed [cells, 85] copy; 128 rows per
   call, 3 calls for up to 384 targets/core), then ~25 small DVE/ACT
   ops: box decode + l1, per-scale positive sums, class softplus sums,
   and the target-class logit correction as a one-hot dot product.

softplus(x) = ln(exp(x) + 1); Exp/Ln/Abs are pinned to the single ACT
table that holds all three (natural_log_exp_and_others) to avoid
per-instruction table reloads.  Sigmoid = 1/(1+exp(-x)) via DVE
reciprocal.  tensor_tensor_reduce is broken on this HW build, so
reductions use multiply + tensor_reduce.
"""

import numpy as np

from concourse import bass, bacc, mybir
from concourse import bass_utils
from concourse.tile import TileContext

F32 = mybir.dt.float32
I32 = mybir.dt.int32

NUM_CLASSES = 80
STAL_GAMMA = np.float32(2.0)
BATCH = 32
NCORES = 8
BPC = BATCH // NCORES          # batches per core
CH = 5 + NUM_CLASSES
HW = (80 * 80, 40 * 40, 20 * 20)
WS = (80, 40, 20)
NCELL = BPC * (HW[0] + HW[1] + HW[2])       # 33600 cells per core
COFF = (0, BPC * HW[0], BPC * (HW[0] + HW[1]))  # per-scale cell offsets
# OBJ stream: per-scale column blocks, scale 2 padded to 128*13
OBJ_COLS = (HW[0] * BPC // 128, HW[1] * BPC // 128, 1664 // 128)  # 200,50,13
NOBJ = HW[0] * BPC + HW[1] * BPC + 1664     # 33664 (64 pad cells of -100)
GROUPS = 3                                  # gather calls (128 targets each)
TPAD = 128 * GROUPS                         # 384; mean load is ~256/core
# meta column layout (GROUPS target-columns per quantity, interleaved)
MC_ADD = 0                                  # (gx, gy)          6 cols
MC_MUL = 6                                  # 1/w x4           12 cols
MC_SUB = 18                                 # (cx, cy, bw, bh) 12 cols
MC_SWM = 30                                 # small_weight/4    3 cols
MC_D0 = 33                                  # obj dedup flags   9 cols
MC_VLD = 42                                 # real-target flag  3 cols
MC_GI = 45                                  # gather row offsets (i32 bits)
MC_OH = 48                                  # class one-hot   240 cols
NMETA = MC_OH + GROUPS * NUM_CLASSES        # 288
# output partial tile column layout
OC_WSP = 0      # class softplus-sum term
OC_OBJ = 1      # 3 cols: per-scale objectness softplus sums
OC_BOX = 4
OC_POS = 5      # 3 cols
OC_CORR = 8
NOUT = 16

_NC_CACHE = None


def _ap(handle_ap, off, dims):
    return bass.AP(handle_ap.tensor, off, [list(d) for d in dims])


def _single_act_table(arch):
    """All of Exp/Ln/Abs live in natural_log_exp_and_others; hide them
    from the other tables so every activation uses one table (one load
    instead of a reload on each Exp<->Ln transition)."""
    tabs = _ORIG_TABLES(arch)
    need = {mybir.ActivationFunctionType.Exp,
            mybir.ActivationFunctionType.Ln}
    out = {}
    for name, fns in tabs.items():
        out[name] = fns if name == "natural_log_exp_and_others" \
            else fns - need
    return out


_ORIG_TABLES = bacc.get_activation_tables


def _build_nc():
    nc = bacc.Bacc("TRN2", target_bir_lowering=False, debug=False)
    fall_t = nc.dram_tensor("FALL", [NCELL * CH], F32, kind="ExternalInput")
    obj_t = nc.dram_tensor("OBJ", [128, sum(OBJ_COLS)], F32,
                           kind="ExternalInput")
    mt_t = nc.dram_tensor("MT", [128, NMETA], F32, kind="ExternalInput")
    out_t = nc.dram_tensor("OUT", [128, NOUT], F32, kind="ExternalOutput")

    EXP = mybir.ActivationFunctionType.Exp
    LN = mybir.ActivationFunctionType.Ln
    AX = mybir.AxisListType
    NOB = sum(OBJ_COLS)
    with TileContext(nc) as tc:
        with tc.tile_pool(name="persist", bufs=1) as pp:
            part = pp.tile([128, NOUT], F32)
            mt = pp.tile([128, NMETA], F32)
            va = pp.tile([128, GROUPS * CH], F32)  # per-target 85-float rows
            vt = pp.tile([128, GROUPS * NUM_CLASSES], F32)
            l1 = pp.tile([128, GROUPS], F32)
            sc = pp.tile([128, GROUPS], F32)
            g3 = pp.tile([128, GROUPS], F32)
            ob = pp.tile([128, NOB], F32)
            # meta (with bit-packed gather offsets) on the scalar HWDGE
            # ring, objectness on the sync ring - they run in parallel
            nc.scalar.dma_start(out=mt[:], in_=mt_t.ap())
            gi = mt[:, MC_GI:MC_GI + GROUPS].bitcast(I32)
            # one 85-float row per target; 128 rows (one per partition)
            # per call; target t sits at (p, j) = (t % 128, t // 128)
            for j in range(GROUPS):
                nc.gpsimd.indirect_dma_start(
                    out=va[:, CH * j:CH * j + CH], out_offset=None,
                    in_=_ap(fall_t.ap(), 0, [[1, NCELL * CH], [1, 1]]),
                    in_offset=bass.IndirectOffsetOnAxis(ap=gi[:, j:j + 1],
                                                        axis=0))

            nc.sync.dma_start(out=ob[:], in_=obj_t.ap())
            nc.vector.memset(part[:], 0.0)

            # ---- dense objectness stream ----
            nc.scalar.activation(ob[:], ob[:], EXP)
            nc.scalar.activation(ob[:], ob[:], LN, bias=1.0)
            ocol = 0
            for s in range(3):
                w = OBJ_COLS[s]
                nc.vector.reduce_sum(part[:, OC_OBJ + s:OC_OBJ + s + 1],
                                     ob[:, ocol:ocol + w], axis=AX.X)
                ocol += w

            # ---- per-target math ----
            va3 = va[:].rearrange("p (j c) -> p j c", c=CH)
            vt3 = vt[:].rearrange("p (j c) -> p j c", c=NUM_CLASSES)
            mt3 = lambda lo, w: mt[:, lo:lo + GROUPS * w].rearrange(
                "p (j c) -> p j c", c=w)
            # box decode: ch0,1 -> sigmoid = 1/(1+exp(-x)) ; ch2,3 ->
            # exp(min(x,4)); one shared EXP pass over ch0..3
            nc.vector.tensor_scalar_mul(va3[:, :, 0:2], va3[:, :, 0:2], -1.0)
            nc.vector.tensor_scalar_min(va3[:, :, 2:4], va3[:, :, 2:4], 4.0)
            nc.scalar.activation(va3[:, :, 0:4], va3[:, :, 0:4], EXP)
            nc.vector.tensor_scalar_add(va3[:, :, 0:2], va3[:, :, 0:2], 1.0)
            nc.vector.reciprocal(va3[:, :, 0:2], va3[:, :, 0:2])
            nc.vector.tensor_mul(va3[:, :, 0:4], va3[:, :, 0:4], mt3(MC_MUL, 4))
            nc.vector.tensor_sub(va3[:, :, 0:4], va3[:, :, 0:4], mt3(MC_SUB, 4))
            nc.vector.reduce_sum(l1[:], va3[:, :, 0:4], axis=AX.X,
                                 apply_absolute_value=True)
            nc.vector.tensor_mul(l1[:], l1[:], mt[:, MC_SWM:MC_SWM + GROUPS])
            nc.vector.reduce_sum(part[:, OC_BOX:OC_BOX + 1], l1[:], axis=AX.X)
            # class-logit correction: one-hot dot with the raw logits
            nc.vector.tensor_mul(vt3, va3[:, :, 5:CH], mt3(MC_OH, NUM_CLASSES))
            nc.vector.reduce_sum(part[:, OC_CORR:OC_CORR + 1], vt[:],
                                 axis=AX.X)
            # objectness positive-cell correction (raw channel 4)
            for s in range(3):
                nc.vector.tensor_mul(
                    sc[:], va3[:, :, 4],
                    mt[:, MC_D0 + GROUPS * s:MC_D0 + GROUPS * s + GROUPS])
                nc.vector.reduce_sum(part[:, OC_POS + s:OC_POS + s + 1],
                                     sc[:], axis=AX.X)
            # class softplus sum over the 80 logits of each target's cell
            nc.scalar.activation(va3[:, :, 5:CH], va3[:, :, 5:CH], EXP)
            nc.scalar.activation(va3[:, :, 5:CH], va3[:, :, 5:CH], LN,
                                 bias=1.0)
            nc.vector.reduce_sum(g3[:], va3[:, :, 5:CH], axis=AX.X)
            nc.vector.tensor_mul(g3[:], g3[:], mt[:, MC_VLD:MC_VLD + GROUPS])
            nc.vector.reduce_sum(part[:, OC_WSP:OC_WSP + 1], g3[:], axis=AX.X)

            nc.sync.dma_start(out=out_t.ap(), in_=part[:])
    bacc.get_activation_tables = _single_act_table
    try:
        nc.compile()
    finally:
        bacc.get_activation_tables = _ORIG_TABLES
    return nc


def get_nc():
    global _NC_CACHE
    if _NC_CACHE is None:
        _NC_CACHE = _build_nc()
    return _NC_CACHE


def prepare_in_maps(pred0, pred1, pred2, targets):
    """Host-side sharding + layout/index preprocessing (numpy only)."""
    preds = (np.asarray(pred0, dtype=np.float32),
             np.asarray(pred1, dtype=np.float32),
             np.asarray(pred2, dtype=np.float32))
    t = np.asarray(targets, dtype=np.float32)
    n = t.shape[0]
    b = t[:, 0].astype(np.int32)
    cls = t[:, 1].astype(np.int32)
    cx, cy, bw, bh = t[:, 2], t[:, 3], t[:, 4], t[:, 5]

    area = np.maximum(bw * bh, np.float32(1e-6))
    s_idx = np.where(area <= 0.01, 0,
                     np.where(area <= 0.03, 1, 2)).astype(np.int32)
    sw = np.float32(1.0) + STAL_GAMMA * (np.float32(1.0) - np.sqrt(area))

    ws = np.array(WS, np.int32)[s_idx]
    wf = ws.astype(np.float32)
    gx = np.clip((cx * wf).astype(np.int32), 0, ws - 1)
    gy = np.clip((cy * wf).astype(np.int32), 0, ws - 1)
    hw = np.array(HW, np.int64)[s_idx]

    b_cl = np.clip(b, 0, BATCH - 1)
    core = b_cl // BPC
    bl = (b_cl % BPC).astype(np.int64)
    cell = (np.array(COFF, np.int64)[s_idx] + bl * hw
            + (gy.astype(np.int64) * ws + gx))

    valid_cls = ((cls >= 0) & (cls < NUM_CLASSES)).astype(np.float32)
    cls_c = np.clip(cls, 0, NUM_CLASSES - 1)

    # obj dedup: one representative target per (scale, batch, gy, gx) cell
    key = ((s_idx.astype(np.int64) * BATCH + b_cl) * 128 + gy) * 128 + gx
    dflag = np.zeros(n, np.float32)
    _, first = np.unique(key, return_index=True)
    dflag[first] = 1.0

    in_maps = []
    for c in range(NCORES):
        sel = np.nonzero(core == c)[0]
        if len(sel) > TPAD:
            sel = sel[:TPAD]  # graceful degradation; never expected
        m = len(sel)
        csel = cell[sel]

        # target t maps to (partition, group) = (t % 128, t // 128)
        ga = np.zeros(TPAD, np.int64)
        ga[:m] = csel * CH

        mt = np.zeros((128, NMETA), np.float32)
        mt[:, MC_GI:MC_GI + GROUPS] = np.ascontiguousarray(
            ga.astype(np.int32).reshape(GROUPS, 128).T).view(np.float32)

        def put(col, vals):
            buf = np.zeros(TPAD, np.float32)
            buf[:m] = vals
            mt[:, col:col + GROUPS] = buf.reshape(GROUPS, 128).T

        def put_il(col, width, *vals):  # channel-interleaved group
            buf = np.zeros((TPAD, width), np.float32)
            for i, v in enumerate(vals):
                buf[:m, i] = v
            mt[:, col:col + GROUPS * width] = buf.reshape(
                GROUPS, 128, width).transpose(1, 0, 2).reshape(
                128, GROUPS * width)

        invw = np.float32(1.0) / wf[sel]
        put_il(MC_MUL, 4, invw, invw, invw, invw)
        put_il(MC_SUB, 4,
               cx[sel] - gx[sel].astype(np.float32) * invw,
               cy[sel] - gy[sel].astype(np.float32) * invw,
               bw[sel], bh[sel])
        put(MC_SWM, sw[sel] * np.float32(0.25))
        for s in range(3):
            put(MC_D0 + GROUPS * s, dflag[sel] * (s_idx[sel] == s))
        put(MC_VLD, np.float32(1.0))
        oh = np.zeros((TPAD, NUM_CLASSES), np.float32)
        oh[np.arange(m), cls_c[sel]] = valid_cls[sel]
        mt[:, MC_OH:] = oh.reshape(GROUPS, 128, NUM_CLASSES).transpose(
            1, 0, 2).reshape(128, GROUPS * NUM_CLASSES)

        lo, hi = c * BPC, (c + 1) * BPC
        fall = np.empty((NCELL, CH), np.float32)
        obj = np.full((128, sum(OBJ_COLS)), np.float32(-100.0), np.float32)
        off = 0
        ocol = 0
        for s, p in enumerate(preds):
            nc_s = BPC * HW[s]
            blk = p[lo:hi].reshape(BPC, CH, HW[s])
            fall[off:off + nc_s] = np.moveaxis(blk, 1, 2).reshape(nc_s, CH)
            w = OBJ_COLS[s]
            tmp = np.full(128 * w, np.float32(-100.0), np.float32)
            tmp[:nc_s] = blk[:, 4].reshape(-1)
            obj[:, ocol:ocol + w] = tmp.reshape(128, w)
            off += nc_s
            ocol += w

        in_maps.append({
            "FALL": fall.reshape(-1),
            "OBJ": obj,
            "MT": mt,
        })
    return in_maps, n


def finalize(results, n):
    """Combine per-core [128, NOUT] partial tiles into the 4 losses."""
    ps = np.stack([np.asarray(r["OUT"], np.float64) for r in results])
    cls_sp = ps[:, :, OC_WSP].sum()
    obj_sp = [ps[:, :, OC_OBJ + s].sum() for s in range(3)]
    box = ps[:, :, OC_BOX].sum()
    pos = [ps[:, :, OC_POS + s].sum() for s in range(3)]
    corr = ps[:, :, OC_CORR].sum()

    norm = max(1, n)
    box_loss = box / norm
    cls_loss = (cls_sp - corr) / (NUM_CLASSES * norm)
    obj_loss = sum((obj_sp[s] - pos[s]) / (BATCH * HW[s]) for s in range(3))
    total = box_loss + obj_loss + cls_loss
    return np.array([total, box_loss, obj_loss, cls_loss], np.float32)


def run_on_hw(in_maps, trace=False):
    nc = get_nc()
    return bass_utils.run_bass_kernel_spmd(
        nc, in_maps, core_ids=list(range(NCORES)), trace=trace)


def kernel(pred0, pred1, pred2, targets, **_unused):
    in_maps, n = prepare_in_maps(pred0, pred1, pred2, targets)
    res = run_on_hw(in_maps)
    return finalize(res.results, n)
